# revision 5
# baseline (speedup 1.0000x reference)
"""EuclidConv + training-mode BatchNorm on 8 Trainium2 NeuronCores.

Math (reference): out = BN(2*conv(x,w) + conv(x^2, ones3x3) + ||w_f||^2),
BN over global-batch stats. The per-filter ||w||^2 term is channel-constant,
so BN's mean subtraction cancels it exactly -> never computed.

Sharding: HYBRID. core c -> (chgrp = c//4, bgrp = c%4): 128 of 256 output
channels x 8 of 32 images. This gives full-width M=128 matmuls (the pure
channel-sharded layout only fills 32 of 128 PE output columns), 4x less PE
streaming. The price: BN statistics must be reduced across the 4 bgrps that
share a channel group -> one tiny 4-rank AllGather of [128,2] partial
(sum, sumsq) + local fold.

Per image m (padded 30x30 grid, fp16):
  u_m = x_m^2                                      (ACT Square)
  r4 psum = ones128.T @ u_m    (channel sums of x^2, replicated over all
                                128 partitions; 2 MMs)
  rc = r4 - 128*validmap       (DVE, fp16, centered for precision)
  box filter: vv = 3-tap vertical (DVE, stride-30 = pair-aligned 2x mode),
  te = vv[0]+vv[+2] (2x), tf = te + vv[+1] (GpSimd - odd offset would be
  1x-mode on DVE anyway, and DVE is the busier engine)
Conv accumulation group per (img, yt-half) [128,392] psum:
  identity.T @ tf_view         (start=True: seeds psum with t1 - 128*count)
  sum_k (2w)_k.T @ x_view      (9 offsets, full M=128)
  ones1.T @ cmap_view          (stop=True: re-adds 128*count)
Drain: ACT copy psum->s_sb with accum S; ACT square with accum Q.
Stats: fold S,Q over 8 local images -> [128,2]; 4-rank AllGather via HBM
bounce; fold 4 ranks; A = gamma*rsqrt(var+eps), B = beta - mean*A;
normalize out = s*A+B (DVE/ACT/GpSimd rotation, fp16) -> DMA out.

Host-side prep is layout/sharding only: pad+transpose+cast of x, weight
transpose/scale, constant masks.
"""
import json

import numpy as np

import concourse.bass as bass
import concourse.mybir as mybir
import concourse.tile as tile
from concourse.ap import AP
from concourse.bass_utils import run_bass_kernel_spmd
from concourse.vector_clock import ScopedClock, VectorClock

F16 = mybir.dt.float16
F32 = mybir.dt.float32

N_CORES = 8
NIMG_L = 8  # images per core
HP = 30
NPIX = HP * HP
NV = 28 * 28
NHW = 32 * NV  # global batch pixels per channel
EPS = 1e-5
CC_GROUPS = [[0, 1, 2, 3], [4, 5, 6, 7]]
N_WARM = 28

_split_ctr = [0]


def _split_waits_json(bir: bytes, max_waits: int = 1) -> bytes:
    """This container's walrus rejects instructions with >1 sync wait.
    Hoist excess waits onto EventSemaphore instructions inserted before the
    offender on the same engine stream."""
    m = json.loads(bir)
    for f in m["functions"]:
        for bb in f["blocks"]:
            newinsts = []
            for ins in bb["instructions"]:
                si = ins.get("sync_info")
                if si:
                    waits = si.get("on_wait") or []
                    if len(waits) > max_waits:
                        extra, keep = waits[:-max_waits], waits[-max_waits:]
                        for w_ in extra:
                            _split_ctr[0] += 1
                            newinsts.append(
                                {
                                    "debug": ins.get("debug", 0),
                                    "engine": ins["engine"],
                                    "ins": [],
                                    "outs": [],
                                    "name": f"antsplitw-{_split_ctr[0]}",
                                    "opcode": "EventSemaphore",
                                    "sync_info": {"on_update": [], "on_wait": [w_]},
                                }
                            )
                        si["on_wait"] = keep
                newinsts.append(ins)
            bb["instructions"] = newinsts
    return json.dumps(m).encode()


class _PatchedBass(bass.Bass):
    def to_json_bytes(self):
        return _split_waits_json(super().to_json_bytes())


class _SplitDrainTileContext(tile.TileContext):
    """Split the tile-exit drain's waits into single-wait drains (same
    walrus limitation as above)."""

    def _drain_and_barrier(self, tick_clock, wait_clock):
        g = tick_clock.global_clock
        n = len(g)
        for i in range(n):
            if g[i] > 0:
                vec = [0] * n
                vec[i] = g[i]
                d = self.nc.sync.drain()
                wait_clock.add_sem_waits(d.ins, ScopedClock({None: VectorClock(vec)}))
        self.nc.sync.drain()
        self.nc.all_engine_barrier()
        assert self.sems is not None
        popped = self.nc._tile_sem_poison_stack.pop()
        assert popped is self._sem_poison
        self.nc.clear_and_free_semaphores(list(self.sems.allocated().values()))
        self.nc.all_engine_barrier()


def _build_nc():
    nc = _PatchedBass(num_devices=N_CORES)
    xh = nc.dram_tensor("xh", [128, NIMG_L * NPIX], F16, kind="ExternalInput")
    wt = nc.dram_tensor("wt", [128, 9 * 128], F16, kind="ExternalInput")
    ones128d = nc.dram_tensor("ones128", [128, 128], F16, kind="ExternalInput")
    id128d = nc.dram_tensor("id128", [128, 128], F16, kind="ExternalInput")
    onesrd = nc.dram_tensor("onesr", [1, 128], F16, kind="ExternalInput")
    comp16d = nc.dram_tensor("comp16", [128, NPIX], F16, kind="ExternalInput")
    cmap16d = nc.dram_tensor("cmap16", [1, 840], F16, kind="ExternalInput")
    cst32d = nc.dram_tensor("cst32", [128, 3], F32, kind="ExternalInput")
    y = nc.dram_tensor("y", [NIMG_L, 128, 28, 28], F16, kind="ExternalOutput")

    with _SplitDrainTileContext(nc) as tc:
        with (
            tc.tile_pool(name="const", bufs=1) as cpool,
            tc.tile_pool(name="xpool", bufs=1) as xpool,
            tc.tile_pool(name="upool", bufs=3) as upool,
            tc.tile_pool(name="boxp", bufs=3) as boxp,
            tc.tile_pool(name="tfp", bufs=4) as tfp,
            tc.tile_pool(name="spool", bufs=1) as spool,
            tc.tile_pool(name="opool", bufs=3) as opool,
            tc.tile_pool(name="psr", bufs=2, space="PSUM") as psr,
            tc.tile_pool(name="psc", bufs=4, space="PSUM") as psc,
            tc.tile_pool(name="dram", bufs=1, space="DRAM") as dram,
        ):
            # ---- dummy collective, triggered first: absorbs the NRT entry
            # barrier + first-collective ncfw setup (~25-50us) under compute,
            # so the real stats AllGather later starts in ~1us ----
            dcin = dram.tile([128, 2], F32, name="dcin")
            dcout = dram.tile([128 * 4, 2], F32, name="dcout")
            nc.gpsimd.collective_compute(
                "AllGather",
                mybir.AluOpType.bypass,
                replica_groups=CC_GROUPS,
                ins=[dcin[:].opt()],
                outs=[dcout[:].opt()],
            )

            # ---- constants (SWDGE queue; x images go on the sync queue) ----
            idt = cpool.tile([128, 128], F16, name="idt")
            nc.gpsimd.dma_start(idt[:], id128d[:])
            ones128 = cpool.tile([128, 128], F16, name="ones128")
            nc.gpsimd.dma_start(ones128[:], ones128d[:])
            wtile = cpool.tile([128, 9 * 128], F16, name="wtile")
            nc.gpsimd.dma_start(wtile[:], wt[:])
            onert = cpool.tile([1, 128], F16, name="onert")
            nc.gpsimd.dma_start(onert[:], onesrd[:])
            compt = cpool.tile([128, NPIX], F16, name="compt")
            nc.gpsimd.dma_start(compt[:], comp16d[:])
            cmapt = cpool.tile([1, 840], F16, name="cmapt")
            nc.gpsimd.dma_start(cmapt[:], cmap16d[:])
            c32 = cpool.tile([128, 3], F32, name="c32")
            nc.gpsimd.dma_start(c32[:], cst32d[:])
            cm3 = cmapt[:].rearrange("p (a c) -> p a c", c=HP)

            xall = xpool.tile([128, NIMG_L * NPIX], F16, name="xall")
            for m in range(NIMG_L):
                nc.sync.dma_start(
                    xall[:, m * NPIX : (m + 1) * NPIX],
                    xh[:, m * NPIX : (m + 1) * NPIX],
                )
            x3 = xall[:].rearrange("p (n a b) -> p n a b", a=HP, b=HP)

            s_sb = spool.tile([128, NIMG_L * NV], F32, name="s_sb")
            sums16 = spool.tile([128, 2 * NIMG_L], F32, name="sums16")
            sumsq = spool.tile([128, NIMG_L], F32, name="sumsq")

            # ---- PE warmup: flip HAM to 8/8 during the input-DMA window ----
            warm = psr.tile([128, 1024], F32, name="warm", tag="r4")
            for i in range(N_WARM):
                nc.tensor.matmul(
                    warm[:, 0:128], idt[:], idt[:], start=True, stop=True,
                    skip_group_check=True,
                )

            # ---- ACT spline-table preload (first activation pays ~1.3us) ----
            tscr = spool.tile([128, 4], F32, name="tscr")
            nc.scalar.activation(
                tscr[:, 0:3], c32[:], mybir.ActivationFunctionType.Square
            )

            # ---- x^2: first two images on DVE (fast startup), rest on ACT;
            # emitted up-front so the ACT FIFO serves them before the drains ----
            uts = []
            for m in range(NIMG_L):
                ut = upool.tile([128, NPIX], F16, name=f"u{m}", tag="u")
                xs = xall[:, m * NPIX : (m + 1) * NPIX]
                if m < 2:
                    nc.vector.tensor_mul(ut[:], xs, xs)
                else:
                    nc.scalar.activation(
                        ut[:], xs, mybir.ActivationFunctionType.Square
                    )
                uts.append(ut)

            tfs = [None] * NIMG_L

            def box_chain(m):
                """r4 matmul + centered cast + separable 3x3 box filter for
                image m; leaves tf (t1 - 128*count on the 30-grid) in tfs[m]."""
                r4 = psr.tile([128, 1024], F32, name=f"r4_{m}", tag="r4")
                for lo, hi in ((0, 512), (512, NPIX)):
                    nc.tensor.matmul(
                        r4[:, lo:hi],
                        ones128[:],
                        uts[m][:, lo:hi],
                        start=True,
                        stop=True,
                        skip_group_check=True,
                    )
                rc = boxp.tile([128, NPIX], F16, name=f"rc{m}", tag="rc")
                nc.vector.tensor_sub(rc[:], r4[:, 0:NPIX], compt[:])
                vv = boxp.tile([128, 840], F16, name=f"vv{m}", tag="vv")
                nc.vector.tensor_add(vv[:], rc[:, 0:840], rc[:, 30:870])
                nc.vector.tensor_add(vv[:], vv[:], rc[:, 60:900])
                te = boxp.tile([128, 840], F16, name=f"te{m}", tag="te")
                nc.vector.tensor_add(te[:, 0:838], vv[:, 0:838], vv[:, 2:840])
                tf = tfp.tile([128, 840], F16, name=f"tf{m}", tag="tf")
                nc.gpsimd.tensor_add(tf[:, 0:838], te[:, 0:838], vv[:, 1:839])
                tfs[m] = tf

            def conv_chunk(b):
                """Conv accumulation groups + drains for images 2b, 2b+1.
                One psum BANK per (img, yt-half) group: finest-grained drain
                so the next chunk's injects never wait long."""
                ms = (2 * b, 2 * b + 1)
                pss = {}
                for m in ms:
                    for yt in range(2):
                        pss[(m, yt)] = psc.tile(
                            [128, 512], F32, name=f"ps{m}_{yt}", tag="ps"
                        )
                # t1 injection (shared idt weights)
                for m in ms:
                    t13 = tfs[m][:].rearrange("p (a c) -> p a c", c=HP)
                    for yt in range(2):
                        nc.tensor.matmul(
                            pss[(m, yt)][:, 0:392],
                            idt[:],
                            t13[:, 14 * yt : 14 * yt + 14, 0:28],
                            start=True,
                            stop=False,
                            skip_group_check=True,
                        )
                # conv: k-major so each weight load serves 4 matmuls
                for k in range(9):
                    dy, dx = divmod(k, 3)
                    for m in ms:
                        for yt in range(2):
                            y0 = yt * 14
                            nc.tensor.matmul(
                                pss[(m, yt)][:, 0:392],
                                wtile[:, k * 128 : (k + 1) * 128],
                                x3[:, m, y0 + dy : y0 + dy + 14, dx : dx + 28],
                                start=False,
                                stop=False,
                                skip_group_check=True,
                            )
                # countmap (uncenter) + close the groups
                for m in ms:
                    for yt in range(2):
                        nc.tensor.matmul(
                            pss[(m, yt)][:, 0:392],
                            onert[:],
                            cm3[:, 14 * yt : 14 * yt + 14, 0:28],
                            start=False,
                            stop=True,
                            skip_group_check=True,
                        )
                # drains: psum -> s_sb with accum S (per half), then squares
                # with accum Q (per image)
                for m in ms:
                    for yt in range(2):
                        blk = m * NV + yt * 392
                        nc.scalar.activation(
                            s_sb[:, blk : blk + 392],
                            pss[(m, yt)][:, 0:392],
                            mybir.ActivationFunctionType.Copy,
                            accum_out=sums16[:, 2 * m + yt : 2 * m + yt + 1],
                        )
                for m in ms:
                    blk = m * NV
                    sq_scr = opool.tile([128, NV], F32, name=f"sq{m}", tag="sq")
                    nc.scalar.activation(
                        sq_scr[:],
                        s_sb[:, blk : blk + NV],
                        mybir.ActivationFunctionType.Square,
                        accum_out=sumsq[:, m : m + 1],
                    )

            for m in (0, 1, 2, 3):
                box_chain(m)
            conv_chunk(0)
            for m in (4, 5):
                box_chain(m)
            conv_chunk(1)
            for m in (6, 7):
                box_chain(m)
            conv_chunk(2)
            conv_chunk(3)

            # ---- stats: local fold -> 4-rank AllGather -> global fold ----
            st2 = spool.tile([128, 2], F32, name="st2")
            nc.vector.tensor_reduce(
                out=st2[:, 0:1], in_=sums16[:], op=mybir.AluOpType.add,
                axis=mybir.AxisListType.X,
            )
            nc.vector.tensor_reduce(
                out=st2[:, 1:2], in_=sumsq[:], op=mybir.AluOpType.add,
                axis=mybir.AxisListType.X,
            )
            cin = dram.tile([128, 2], F32, name="cin")
            cout = dram.tile([128 * 4, 2], F32, name="cout")
            nc.gpsimd.dma_start(cin[:], st2[:])
            nc.gpsimd.collective_compute(
                "AllGather",
                mybir.AluOpType.bypass,
                replica_groups=CC_GROUPS,
                ins=[cin[:].opt()],
                outs=[cout[:].opt()],
            )
            g = spool.tile([128, 8], F32, name="g")
            nc.sync.dma_start(
                g[:], AP(cout.tensor, cout.offset, [[2, 128], [256, 4], [1, 2]])
            )
            gs = spool.tile([128, 2], F32, name="gs")
            nc.vector.tensor_add(gs[:], g[:, 0:2], g[:, 2:4])
            nc.vector.tensor_add(gs[:], gs[:], g[:, 4:6])
            nc.vector.tensor_add(gs[:], gs[:], g[:, 6:8])

            ab = spool.tile([128, 8], F32, name="ab")
            mean = ab[:, 0:1]
            qn = ab[:, 1:2]
            nc.vector.tensor_scalar_mul(mean, gs[:, 0:1], 1.0 / NHW)
            nc.vector.tensor_scalar_mul(qn, gs[:, 1:2], 1.0 / NHW)
            var = ab[:, 2:3]
            nc.vector.scalar_tensor_tensor(
                var, mean, 1.0, mean, op0=mybir.AluOpType.mult,
                op1=mybir.AluOpType.mult,
            )
            nc.vector.tensor_sub(var, qn, var)
            sd = ab[:, 3:4]
            nc.scalar.activation(
                sd, var, mybir.ActivationFunctionType.Sqrt, bias=c32[:, 2:3]
            )
            abv = spool.tile([128, 2], F32, name="abv")
            A = abv[:, 0:1]
            B = abv[:, 1:2]
            nc.vector.reciprocal(A, sd)
            nc.vector.tensor_mul(A, A, c32[:, 0:1])
            nc.vector.scalar_tensor_tensor(
                B, mean, 1.0, A, op0=mybir.AluOpType.mult, op1=mybir.AluOpType.mult
            )
            nc.vector.tensor_sub(B, c32[:, 1:2], B)

            # ---- normalize + store (engine rotation) ----
            for m in range(NIMG_L):
                blk = m * NV
                o = opool.tile([128, NV], F16, name=f"o{m}", tag="o")
                if m % 2 == 0:
                    nc.vector.tensor_scalar(
                        o[:],
                        s_sb[:, blk : blk + NV],
                        A,
                        B,
                        op0=mybir.AluOpType.mult,
                        op1=mybir.AluOpType.add,
                    )
                else:
                    nc.scalar.activation(
                        o[:],
                        s_sb[:, blk : blk + NV],
                        mybir.ActivationFunctionType.Identity,
                        bias=B,
                        scale=A,
                    )
                dst = AP(y.ap().tensor, m * 128 * NV, [[NV, 128], [1, NV]])
                eng = nc.sync if m % 2 == 0 else nc.scalar
                eng.dma_start(dst, o[:])
    return nc


def _prep_inputs(x, w, gamma, beta):
    x = np.asarray(x, np.float32)
    w = np.asarray(w, np.float32)
    gamma = np.asarray(gamma, np.float32)
    beta = np.asarray(beta, np.float32)

    xp = np.zeros((32, 128, HP, HP), np.float32)
    xp[:, :, 1:29, 1:29] = x

    ones128 = np.ones((128, 128), np.float16)
    id128 = np.eye(128, dtype=np.float16)
    onesr = np.ones((1, 128), np.float16)

    pidx = np.arange(NPIX)
    py, px = pidx // HP, pidx % HP
    valid = (py >= 1) & (py <= 28) & (px >= 1) & (px <= 28)
    comp16 = np.broadcast_to((128.0 * valid).astype(np.float16), (128, NPIX)).copy()

    jj = np.arange(840)
    jy, jx = jj // HP, jj % HP
    cy = np.where((jy == 0) | (jy == 27), 2, 3)
    cx = np.where((jx == 0) | (jx == 27), 2, 3)
    used = (jy < 28) & (jx < 28)
    cmap16 = np.where(used, 128.0 * cy * cx, 0.0).astype(np.float16)[None, :]

    maps = []
    for core in range(N_CORES):
        cg, bg = core // 4, core % 4
        xs = xp[bg * NIMG_L : (bg + 1) * NIMG_L]
        xhc = np.ascontiguousarray(xs.transpose(1, 0, 2, 3)).reshape(
            128, NIMG_L * NPIX
        )
        wc = (2.0 * w[cg * 128 : (cg + 1) * 128]).reshape(128, 128, 9)
        wtc = np.ascontiguousarray(wc.transpose(1, 2, 0)).reshape(128, 9 * 128)
        cst32 = np.zeros((128, 3), np.float32)
        cst32[:, 0] = gamma[cg * 128 : (cg + 1) * 128]
        cst32[:, 1] = beta[cg * 128 : (cg + 1) * 128]
        cst32[:, 2] = EPS
        maps.append(
            {
                "xh": xhc.astype(np.float16),
                "wt": wtc.astype(np.float16),
                "ones128": ones128,
                "id128": id128,
                "onesr": onesr,
                "comp16": comp16,
                "cmap16": cmap16,
                "cst32": cst32,
            }
        )
    return maps


_NC_CACHE = []


def _assemble(results):
    out = np.empty((32, 256, 28, 28), np.float32)
    for core in range(N_CORES):
        cg, bg = core // 4, core % 4
        out[bg * NIMG_L : (bg + 1) * NIMG_L, cg * 128 : (cg + 1) * 128] = (
            results[core]["y"].astype(np.float32)
        )
    return out


def kernel(x, w, gamma, beta):
    if not _NC_CACHE:
        _NC_CACHE.append(_build_nc())
    nc = _NC_CACHE[0]
    maps = _prep_inputs(x, w, gamma, beta)
    res = run_bass_kernel_spmd(nc, maps, core_ids=list(range(N_CORES)))
    return _assemble(res.results)


# revision 7
# speedup vs baseline: 1.0848x; 1.0848x over previous
"""EuclidConv + training-mode BatchNorm on 8 Trainium2 NeuronCores.

Math (reference): out = BN(2*conv(x,w) + conv(x^2, ones3x3) + ||w_f||^2),
BN over global-batch stats. The per-filter ||w||^2 term is channel-constant,
so BN's mean subtraction cancels it exactly -> never computed.

Sharding: HYBRID. core c -> (chgrp = c//4, bgrp = c%4): 128 of 256 output
channels x 8 of 32 images. This gives full-width M=128 matmuls (the pure
channel-sharded layout only fills 32 of 128 PE output columns), 4x less PE
streaming. The price: BN statistics must be reduced across the 4 bgrps that
share a channel group -> one tiny 4-rank AllGather of [128,2] partial
(sum, sumsq) + local fold.

Per image m (padded 30x30 grid, fp16):
  u_m = x_m^2                                      (ACT Square)
  r4 psum = ones128.T @ u_m    (channel sums of x^2, replicated over all
                                128 partitions; 2 MMs)
  rc = r4 - 128*validmap       (DVE, fp16, centered for precision)
  box filter: vv = 3-tap vertical (DVE, stride-30 = pair-aligned 2x mode),
  te = vv[0]+vv[+2] (2x), tf = te + vv[+1] (GpSimd - odd offset would be
  1x-mode on DVE anyway, and DVE is the busier engine)
Conv accumulation group per (img, yt-half) [128,392] psum:
  identity.T @ tf_view         (start=True: seeds psum with t1 - 128*count)
  sum_k (2w)_k.T @ x_view      (9 offsets, full M=128)
  ones1.T @ cmap_view          (stop=True: re-adds 128*count)
Drain: ACT copy psum->s_sb with accum S; ACT square with accum Q.
Stats: fold S,Q over 8 local images -> [128,2]; 4-rank AllGather via HBM
bounce; fold 4 ranks; A = gamma*rsqrt(var+eps), B = beta - mean*A;
normalize out = s*A+B (DVE/ACT/GpSimd rotation, fp16) -> DMA out.

Host-side prep is layout/sharding only: pad+transpose+cast of x, weight
transpose/scale, constant masks.
"""
import json

import numpy as np

import concourse.bass as bass
import concourse.mybir as mybir
import concourse.tile as tile
from concourse.ap import AP
from concourse.bass_utils import run_bass_kernel_spmd
from concourse.vector_clock import ScopedClock, VectorClock

F16 = mybir.dt.float16
F32 = mybir.dt.float32

N_CORES = 8
NIMG_L = 8  # images per core
HP = 30
NPIX = HP * HP
NV = 28 * 28
NHW = 32 * NV  # global batch pixels per channel
EPS = 1e-5
CC_GROUPS = [[0, 1, 2, 3], [4, 5, 6, 7]]
N_WARM = 28

_split_ctr = [0]


def _split_waits_json(bir: bytes, max_waits: int = 1) -> bytes:
    """This container's walrus rejects instructions with >1 sync wait.
    Hoist excess waits onto EventSemaphore instructions inserted before the
    offender on the same engine stream."""
    m = json.loads(bir)
    for f in m["functions"]:
        for bb in f["blocks"]:
            newinsts = []
            for ins in bb["instructions"]:
                si = ins.get("sync_info")
                if si:
                    waits = si.get("on_wait") or []
                    if len(waits) > max_waits:
                        extra, keep = waits[:-max_waits], waits[-max_waits:]
                        for w_ in extra:
                            _split_ctr[0] += 1
                            newinsts.append(
                                {
                                    "debug": ins.get("debug", 0),
                                    "engine": ins["engine"],
                                    "ins": [],
                                    "outs": [],
                                    "name": f"antsplitw-{_split_ctr[0]}",
                                    "opcode": "EventSemaphore",
                                    "sync_info": {"on_update": [], "on_wait": [w_]},
                                }
                            )
                        si["on_wait"] = keep
                newinsts.append(ins)
            bb["instructions"] = newinsts
    return json.dumps(m).encode()


class _PatchedBass(bass.Bass):
    def to_json_bytes(self):
        return _split_waits_json(super().to_json_bytes())


class _SplitDrainTileContext(tile.TileContext):
    """Split the tile-exit drain's waits into single-wait drains (same
    walrus limitation as above)."""

    def _drain_and_barrier(self, tick_clock, wait_clock):
        g = tick_clock.global_clock
        n = len(g)
        for i in range(n):
            if g[i] > 0:
                vec = [0] * n
                vec[i] = g[i]
                d = self.nc.sync.drain()
                wait_clock.add_sem_waits(d.ins, ScopedClock({None: VectorClock(vec)}))
        self.nc.sync.drain()
        self.nc.all_engine_barrier()
        assert self.sems is not None
        popped = self.nc._tile_sem_poison_stack.pop()
        assert popped is self._sem_poison
        self.nc.clear_and_free_semaphores(list(self.sems.allocated().values()))
        self.nc.all_engine_barrier()


def _build_nc():
    nc = _PatchedBass(num_devices=N_CORES)
    xh = nc.dram_tensor("xh", [128, NIMG_L * NPIX], F16, kind="ExternalInput")
    wt = nc.dram_tensor("wt", [128, 9 * 128], F16, kind="ExternalInput")
    ones128d = nc.dram_tensor("ones128", [128, 128], F16, kind="ExternalInput")
    id128d = nc.dram_tensor("id128", [128, 128], F16, kind="ExternalInput")
    onesrd = nc.dram_tensor("onesr", [1, 128], F16, kind="ExternalInput")
    comp16d = nc.dram_tensor("comp16", [128, NPIX], F16, kind="ExternalInput")
    cmap16d = nc.dram_tensor("cmap16", [1, 840], F16, kind="ExternalInput")
    cst32d = nc.dram_tensor("cst32", [128, 3], F32, kind="ExternalInput")
    y = nc.dram_tensor("y", [NIMG_L, 128, 28, 28], F16, kind="ExternalOutput")

    with _SplitDrainTileContext(nc) as tc:
        with (
            tc.tile_pool(name="const", bufs=1) as cpool,
            tc.tile_pool(name="xpool", bufs=1) as xpool,
            tc.tile_pool(name="upool", bufs=3) as upool,
            tc.tile_pool(name="boxp", bufs=3) as boxp,
            tc.tile_pool(name="tfp", bufs=4) as tfp,
            tc.tile_pool(name="spool", bufs=1) as spool,
            tc.tile_pool(name="opool", bufs=8) as opool,
            tc.tile_pool(name="psr", bufs=2, space="PSUM") as psr,
            tc.tile_pool(name="psc", bufs=4, space="PSUM") as psc,
            tc.tile_pool(name="dram", bufs=1, space="DRAM") as dram,
        ):
            # ---- dummy collective, triggered first: absorbs the NRT entry
            # barrier + first-collective ncfw setup (~25-50us) under compute,
            # so the real stats AllGather later starts in ~1us ----
            dcin = dram.tile([128, 2], F32, name="dcin")
            dcout = dram.tile([128 * 4, 2], F32, name="dcout")
            nc.gpsimd.collective_compute(
                "AllGather",
                mybir.AluOpType.bypass,
                replica_groups=CC_GROUPS,
                ins=[dcin[:].opt()],
                outs=[dcout[:].opt()],
            )

            # ---- constants (SWDGE queue; x images go on the sync queue) ----
            idt = cpool.tile([128, 128], F16, name="idt")
            nc.gpsimd.dma_start(idt[:], id128d[:])
            ones128 = cpool.tile([128, 128], F16, name="ones128")
            nc.gpsimd.dma_start(ones128[:], ones128d[:])
            wtile = cpool.tile([128, 9 * 128], F16, name="wtile")
            nc.gpsimd.dma_start(wtile[:], wt[:])
            onert = cpool.tile([1, 128], F16, name="onert")
            nc.gpsimd.dma_start(onert[:], onesrd[:])
            compt = cpool.tile([128, NPIX], F16, name="compt")
            nc.gpsimd.dma_start(compt[:], comp16d[:])
            cmapt = cpool.tile([1, 840], F16, name="cmapt")
            nc.gpsimd.dma_start(cmapt[:], cmap16d[:])
            c32 = cpool.tile([128, 3], F32, name="c32")
            nc.gpsimd.dma_start(c32[:], cst32d[:])
            cm3 = cmapt[:].rearrange("p (a c) -> p a c", c=HP)

            xall = xpool.tile([128, NIMG_L * NPIX], F16, name="xall")
            for m in range(NIMG_L):
                nc.sync.dma_start(
                    xall[:, m * NPIX : (m + 1) * NPIX],
                    xh[:, m * NPIX : (m + 1) * NPIX],
                )
            x3 = xall[:].rearrange("p (n a b) -> p n a b", a=HP, b=HP)

            s_sb = spool.tile([128, NIMG_L * NV], F32, name="s_sb")
            sums16 = spool.tile([128, 2 * NIMG_L], F32, name="sums16")
            sumsq = spool.tile([128, NIMG_L], F32, name="sumsq")

            # ---- PE warmup: flip HAM to 8/8 during the input-DMA window ----
            warm = psr.tile([128, 1024], F32, name="warm", tag="r4")
            for i in range(N_WARM):
                nc.tensor.matmul(
                    warm[:, 0:128], idt[:], idt[:], start=True, stop=True,
                    skip_group_check=True,
                )

            # ---- ACT spline-table preload (first activation pays ~1.3us) ----
            tscr = spool.tile([128, 4], F32, name="tscr")
            nc.scalar.activation(
                tscr[:, 0:3], c32[:], mybir.ActivationFunctionType.Square
            )

            # ---- x^2: first two images on DVE (fast startup), rest on ACT;
            # emitted up-front so the ACT FIFO serves them before the drains ----
            uts = []
            for m in range(NIMG_L):
                ut = upool.tile([128, NPIX], F16, name=f"u{m}", tag="u")
                xs = xall[:, m * NPIX : (m + 1) * NPIX]
                if m < 2:
                    nc.vector.tensor_mul(ut[:], xs, xs)
                else:
                    nc.scalar.activation(
                        ut[:], xs, mybir.ActivationFunctionType.Square
                    )
                uts.append(ut)

            tfs = [None] * NIMG_L

            def box_chain(m):
                """r4 matmul + centered cast + separable 3x3 box filter for
                image m; leaves tf (t1 - 128*count on the 30-grid) in tfs[m]."""
                r4 = psr.tile([128, 1024], F32, name=f"r4_{m}", tag="r4")
                for lo, hi in ((0, 512), (512, NPIX)):
                    nc.tensor.matmul(
                        r4[:, lo:hi],
                        ones128[:],
                        uts[m][:, lo:hi],
                        start=True,
                        stop=True,
                        skip_group_check=True,
                    )
                rc = boxp.tile([128, NPIX], F16, name=f"rc{m}", tag="rc")
                nc.vector.tensor_sub(rc[:], r4[:, 0:NPIX], compt[:])
                vv = boxp.tile([128, 840], F16, name=f"vv{m}", tag="vv")
                nc.vector.tensor_add(vv[:], rc[:, 0:840], rc[:, 30:870])
                nc.vector.tensor_add(vv[:], vv[:], rc[:, 60:900])
                te = boxp.tile([128, 840], F16, name=f"te{m}", tag="te")
                nc.vector.tensor_add(te[:, 0:838], vv[:, 0:838], vv[:, 2:840])
                tf = tfp.tile([128, 840], F16, name=f"tf{m}", tag="tf")
                eng = nc.gpsimd if m % 2 == 0 else nc.vector
                eng.tensor_add(tf[:, 0:838], te[:, 0:838], vv[:, 1:839])
                tfs[m] = tf

            def conv_chunk(b):
                """Conv accumulation groups + drains for images 2b, 2b+1.
                One psum BANK per (img, yt-half) group: finest-grained drain
                so the next chunk's injects never wait long."""
                ms = (2 * b, 2 * b + 1)
                pss = {}
                for m in ms:
                    for yt in range(2):
                        pss[(m, yt)] = psc.tile(
                            [128, 512], F32, name=f"ps{m}_{yt}", tag="ps"
                        )
                # t1 injection (shared idt weights)
                for m in ms:
                    t13 = tfs[m][:].rearrange("p (a c) -> p a c", c=HP)
                    for yt in range(2):
                        nc.tensor.matmul(
                            pss[(m, yt)][:, 0:392],
                            idt[:],
                            t13[:, 14 * yt : 14 * yt + 14, 0:28],
                            start=True,
                            stop=False,
                            skip_group_check=True,
                        )
                # conv: k-major so each weight load serves 4 matmuls
                for k in range(9):
                    dy, dx = divmod(k, 3)
                    for m in ms:
                        for yt in range(2):
                            y0 = yt * 14
                            nc.tensor.matmul(
                                pss[(m, yt)][:, 0:392],
                                wtile[:, k * 128 : (k + 1) * 128],
                                x3[:, m, y0 + dy : y0 + dy + 14, dx : dx + 28],
                                start=False,
                                stop=False,
                                skip_group_check=True,
                            )
                # countmap (uncenter) + close the groups
                for m in ms:
                    for yt in range(2):
                        nc.tensor.matmul(
                            pss[(m, yt)][:, 0:392],
                            onert[:],
                            cm3[:, 14 * yt : 14 * yt + 14, 0:28],
                            start=False,
                            stop=True,
                            skip_group_check=True,
                        )
                # drains: psum -> s_sb with accum S (per half), then squares
                # with accum Q (per image)
                for m in ms:
                    for yt in range(2):
                        blk = m * NV + yt * 392
                        nc.scalar.activation(
                            s_sb[:, blk : blk + 392],
                            pss[(m, yt)][:, 0:392],
                            mybir.ActivationFunctionType.Copy,
                            accum_out=sums16[:, 2 * m + yt : 2 * m + yt + 1],
                        )
                for m in ms:
                    blk = m * NV
                    sq_scr = opool.tile([128, NV], F32, name=f"sq{m}", tag="sq")
                    nc.scalar.activation(
                        sq_scr[:],
                        s_sb[:, blk : blk + NV],
                        mybir.ActivationFunctionType.Square,
                        accum_out=sumsq[:, m : m + 1],
                    )

            for m in (0, 1, 2, 3):
                box_chain(m)
            conv_chunk(0)
            for m in (4, 5):
                box_chain(m)
            conv_chunk(1)
            for m in (6, 7):
                box_chain(m)
            conv_chunk(2)
            conv_chunk(3)

            # ---- stats: local fold -> 4-rank AllGather -> global fold ----
            st2 = spool.tile([128, 2], F32, name="st2")
            nc.vector.tensor_reduce(
                out=st2[:, 0:1], in_=sums16[:], op=mybir.AluOpType.add,
                axis=mybir.AxisListType.X,
            )
            nc.vector.tensor_reduce(
                out=st2[:, 1:2], in_=sumsq[:], op=mybir.AluOpType.add,
                axis=mybir.AxisListType.X,
            )
            cin = dram.tile([128, 2], F32, name="cin")
            cout = dram.tile([128 * 4, 2], F32, name="cout")
            nc.sync.dma_start(cin[:], st2[:])
            nc.gpsimd.collective_compute(
                "AllGather",
                mybir.AluOpType.bypass,
                replica_groups=CC_GROUPS,
                ins=[cin[:].opt()],
                outs=[cout[:].opt()],
            )
            g = spool.tile([128, 8], F32, name="g")
            nc.sync.dma_start(
                g[:], AP(cout.tensor, cout.offset, [[2, 128], [256, 4], [1, 2]])
            )
            gs = spool.tile([128, 2], F32, name="gs")
            nc.vector.tensor_add(gs[:], g[:, 0:2], g[:, 2:4])
            nc.vector.tensor_add(gs[:], gs[:], g[:, 4:6])
            nc.vector.tensor_add(gs[:], gs[:], g[:, 6:8])

            ab = spool.tile([128, 8], F32, name="ab")
            mean = ab[:, 0:1]
            qn = ab[:, 1:2]
            nc.vector.tensor_scalar_mul(mean, gs[:, 0:1], 1.0 / NHW)
            nc.vector.tensor_scalar_mul(qn, gs[:, 1:2], 1.0 / NHW)
            var = ab[:, 2:3]
            nc.vector.scalar_tensor_tensor(
                var, mean, 1.0, mean, op0=mybir.AluOpType.mult,
                op1=mybir.AluOpType.mult,
            )
            nc.vector.tensor_sub(var, qn, var)
            sd = ab[:, 3:4]
            nc.scalar.activation(
                sd, var, mybir.ActivationFunctionType.Sqrt, bias=c32[:, 2:3]
            )
            abv = spool.tile([128, 2], F32, name="abv")
            A = abv[:, 0:1]
            B = abv[:, 1:2]
            nc.vector.reciprocal(A, sd)
            nc.vector.tensor_mul(A, A, c32[:, 0:1])
            nc.vector.scalar_tensor_tensor(
                B, mean, 1.0, A, op0=mybir.AluOpType.mult, op1=mybir.AluOpType.mult
            )
            nc.vector.tensor_sub(B, c32[:, 1:2], B)

            # ---- normalize + store (engine rotation) ----
            for m in range(NIMG_L):
                blk = m * NV
                o = opool.tile([128, NV], F16, name=f"o{m}", tag="o")
                if m % 2 == 0:
                    nc.vector.tensor_scalar(
                        o[:],
                        s_sb[:, blk : blk + NV],
                        A,
                        B,
                        op0=mybir.AluOpType.mult,
                        op1=mybir.AluOpType.add,
                    )
                else:
                    nc.scalar.activation(
                        o[:],
                        s_sb[:, blk : blk + NV],
                        mybir.ActivationFunctionType.Identity,
                        bias=B,
                        scale=A,
                    )
                dst = AP(y.ap().tensor, m * 128 * NV, [[NV, 128], [1, NV]])
                eng = nc.sync if m % 2 == 0 else nc.scalar
                eng.dma_start(dst, o[:])
    return nc


def _prep_inputs(x, w, gamma, beta):
    x = np.asarray(x, np.float32)
    w = np.asarray(w, np.float32)
    gamma = np.asarray(gamma, np.float32)
    beta = np.asarray(beta, np.float32)

    xp = np.zeros((32, 128, HP, HP), np.float32)
    xp[:, :, 1:29, 1:29] = x

    ones128 = np.ones((128, 128), np.float16)
    id128 = np.eye(128, dtype=np.float16)
    onesr = np.ones((1, 128), np.float16)

    pidx = np.arange(NPIX)
    py, px = pidx // HP, pidx % HP
    valid = (py >= 1) & (py <= 28) & (px >= 1) & (px <= 28)
    comp16 = np.broadcast_to((128.0 * valid).astype(np.float16), (128, NPIX)).copy()

    jj = np.arange(840)
    jy, jx = jj // HP, jj % HP
    cy = np.where((jy == 0) | (jy == 27), 2, 3)
    cx = np.where((jx == 0) | (jx == 27), 2, 3)
    used = (jy < 28) & (jx < 28)
    cmap16 = np.where(used, 128.0 * cy * cx, 0.0).astype(np.float16)[None, :]

    maps = []
    for core in range(N_CORES):
        cg, bg = core // 4, core % 4
        xs = xp[bg * NIMG_L : (bg + 1) * NIMG_L]
        xhc = np.ascontiguousarray(xs.transpose(1, 0, 2, 3)).reshape(
            128, NIMG_L * NPIX
        )
        wc = (2.0 * w[cg * 128 : (cg + 1) * 128]).reshape(128, 128, 9)
        wtc = np.ascontiguousarray(wc.transpose(1, 2, 0)).reshape(128, 9 * 128)
        cst32 = np.zeros((128, 3), np.float32)
        cst32[:, 0] = gamma[cg * 128 : (cg + 1) * 128]
        cst32[:, 1] = beta[cg * 128 : (cg + 1) * 128]
        cst32[:, 2] = EPS
        maps.append(
            {
                "xh": xhc.astype(np.float16),
                "wt": wtc.astype(np.float16),
                "ones128": ones128,
                "id128": id128,
                "onesr": onesr,
                "comp16": comp16,
                "cmap16": cmap16,
                "cst32": cst32,
            }
        )
    return maps


_NC_CACHE = []


def _assemble(results):
    out = np.empty((32, 256, 28, 28), np.float32)
    for core in range(N_CORES):
        cg, bg = core // 4, core % 4
        out[bg * NIMG_L : (bg + 1) * NIMG_L, cg * 128 : (cg + 1) * 128] = (
            results[core]["y"].astype(np.float32)
        )
    return out


def kernel(x, w, gamma, beta):
    if not _NC_CACHE:
        _NC_CACHE.append(_build_nc())
    nc = _NC_CACHE[0]
    maps = _prep_inputs(x, w, gamma, beta)
    res = run_bass_kernel_spmd(nc, maps, core_ids=list(range(N_CORES)))
    return _assemble(res.results)


# revision 8
# speedup vs baseline: 1.1106x; 1.0238x over previous
"""EuclidConv + training-mode BatchNorm on 8 Trainium2 NeuronCores.

Math (reference): out = BN(2*conv(x,w) + conv(x^2, ones3x3) + ||w_f||^2),
BN over global-batch stats. The per-filter ||w||^2 term is channel-constant,
so BN's mean subtraction cancels it exactly -> never computed.

Sharding: HYBRID. core c -> (chgrp = c//4, bgrp = c%4): 128 of 256 output
channels x 8 of 32 images. This gives full-width M=128 matmuls (the pure
channel-sharded layout only fills 32 of 128 PE output columns), 4x less PE
streaming. The price: BN statistics must be reduced across the 4 bgrps that
share a channel group -> one tiny 4-rank AllGather of [128,2] partial
(sum, sumsq) + local fold.

Per image m (padded 30x30 grid, fp16):
  u_m = x_m^2                                      (ACT Square)
  r4 psum = ones128.T @ u_m    (channel sums of x^2, replicated over all
                                128 partitions; 2 MMs)
  rc = r4 - 128*validmap       (DVE, fp16, centered for precision)
  box filter: vv = 3-tap vertical (DVE, stride-30 = pair-aligned 2x mode),
  te = vv[0]+vv[+2] (2x), tf = te + vv[+1] (GpSimd - odd offset would be
  1x-mode on DVE anyway, and DVE is the busier engine)
Conv accumulation group per (img, yt-half) [128,392] psum:
  identity.T @ tf_view         (start=True: seeds psum with t1 - 128*count)
  sum_k (2w)_k.T @ x_view      (9 offsets, full M=128)
  ones1.T @ cmap_view          (stop=True: re-adds 128*count)
Drain: ACT copy psum->s_sb with accum S; ACT square with accum Q.
Stats: fold S,Q over 8 local images -> [128,2]; 4-rank AllGather via HBM
bounce; fold 4 ranks; A = gamma*rsqrt(var+eps), B = beta - mean*A;
normalize out = s*A+B (DVE/ACT/GpSimd rotation, fp16) -> DMA out.

Host-side prep is layout/sharding only: pad+transpose+cast of x, weight
transpose/scale, constant masks.
"""
import json

import numpy as np

import concourse.bass as bass
import concourse.mybir as mybir
import concourse.tile as tile
from concourse.ap import AP
from concourse.bass_utils import run_bass_kernel_spmd
from concourse.vector_clock import ScopedClock, VectorClock

F16 = mybir.dt.float16
F32 = mybir.dt.float32

N_CORES = 8
NIMG_L = 8  # images per core
HP = 30
NPIX = HP * HP
NV = 28 * 28
NHW = 32 * NV  # global batch pixels per channel
EPS = 1e-5
CC_GROUPS = [[0, 1, 2, 3], [4, 5, 6, 7]]
N_WARM = 28

_split_ctr = [0]


def _split_waits_json(bir: bytes, max_waits: int = 1) -> bytes:
    """This container's walrus rejects instructions with >1 sync wait.
    Hoist excess waits onto EventSemaphore instructions inserted before the
    offender on the same engine stream."""
    m = json.loads(bir)
    for f in m["functions"]:
        for bb in f["blocks"]:
            newinsts = []
            for ins in bb["instructions"]:
                si = ins.get("sync_info")
                if si:
                    waits = si.get("on_wait") or []
                    if len(waits) > max_waits:
                        extra, keep = waits[:-max_waits], waits[-max_waits:]
                        for w_ in extra:
                            _split_ctr[0] += 1
                            newinsts.append(
                                {
                                    "debug": ins.get("debug", 0),
                                    "engine": ins["engine"],
                                    "ins": [],
                                    "outs": [],
                                    "name": f"antsplitw-{_split_ctr[0]}",
                                    "opcode": "EventSemaphore",
                                    "sync_info": {"on_update": [], "on_wait": [w_]},
                                }
                            )
                        si["on_wait"] = keep
                newinsts.append(ins)
            bb["instructions"] = newinsts
    return json.dumps(m).encode()


class _PatchedBass(bass.Bass):
    def to_json_bytes(self):
        return _split_waits_json(super().to_json_bytes())


class _SplitDrainTileContext(tile.TileContext):
    """Split the tile-exit drain's waits into single-wait drains (same
    walrus limitation as above)."""

    def _drain_and_barrier(self, tick_clock, wait_clock):
        g = tick_clock.global_clock
        n = len(g)
        for i in range(n):
            if g[i] > 0:
                vec = [0] * n
                vec[i] = g[i]
                d = self.nc.sync.drain()
                wait_clock.add_sem_waits(d.ins, ScopedClock({None: VectorClock(vec)}))
        self.nc.sync.drain()
        self.nc.all_engine_barrier()
        assert self.sems is not None
        popped = self.nc._tile_sem_poison_stack.pop()
        assert popped is self._sem_poison
        self.nc.clear_and_free_semaphores(list(self.sems.allocated().values()))
        self.nc.all_engine_barrier()


def _build_nc():
    nc = _PatchedBass(num_devices=N_CORES)
    xh = nc.dram_tensor("xh", [128, NIMG_L * NPIX], F16, kind="ExternalInput")
    wt = nc.dram_tensor("wt", [128, 9 * 128], F16, kind="ExternalInput")
    ones128d = nc.dram_tensor("ones128", [128, 128], F16, kind="ExternalInput")
    id128d = nc.dram_tensor("id128", [128, 128], F16, kind="ExternalInput")
    onesrd = nc.dram_tensor("onesr", [1, 128], F16, kind="ExternalInput")
    comp16d = nc.dram_tensor("comp16", [128, NPIX], F16, kind="ExternalInput")
    cmap16d = nc.dram_tensor("cmap16", [1, 840], F16, kind="ExternalInput")
    cst32d = nc.dram_tensor("cst32", [128, 3], F32, kind="ExternalInput")
    y = nc.dram_tensor("y", [NIMG_L, 128, 28, 28], F16, kind="ExternalOutput")

    with _SplitDrainTileContext(nc) as tc:
        with (
            tc.tile_pool(name="const", bufs=1) as cpool,
            tc.tile_pool(name="xpool", bufs=1) as xpool,
            tc.tile_pool(name="upool", bufs=3) as upool,
            tc.tile_pool(name="boxp", bufs=3) as boxp,
            tc.tile_pool(name="tfp", bufs=4) as tfp,
            tc.tile_pool(name="spool", bufs=1) as spool,
            tc.tile_pool(name="opool", bufs=8) as opool,
            tc.tile_pool(name="psr", bufs=2, space="PSUM") as psr,
            tc.tile_pool(name="psc", bufs=4, space="PSUM") as psc,
            tc.tile_pool(name="dram", bufs=1, space="DRAM") as dram,
        ):
            # ---- dummy collective, triggered first: absorbs the NRT entry
            # barrier + first-collective ncfw setup (~25-50us) under compute,
            # so the real stats AllGather later starts in ~1us ----
            dcin = dram.tile([128, 2], F32, name="dcin")
            dcout = dram.tile([128 * 4, 2], F32, name="dcout")
            nc.gpsimd.collective_compute(
                "AllGather",
                mybir.AluOpType.bypass,
                replica_groups=CC_GROUPS,
                ins=[dcin[:].opt()],
                outs=[dcout[:].opt()],
            )

            # ---- constants: criticality-ordered. Early consumers (warmup,
            # r4, rc) load on the sync queue ahead of the images; bulky /
            # late-consumed ones go on the gpsimd queue ----
            idt = cpool.tile([128, 128], F16, name="idt")
            nc.sync.dma_start(idt[:], id128d[:])
            ones128 = cpool.tile([128, 128], F16, name="ones128")
            nc.sync.dma_start(ones128[:], ones128d[:])
            compt = cpool.tile([128, NPIX], F16, name="compt")
            nc.sync.dma_start(compt[:], comp16d[:])
            c32 = cpool.tile([128, 3], F32, name="c32")
            nc.sync.dma_start(c32[:], cst32d[:])
            wtile = cpool.tile([128, 9 * 128], F16, name="wtile")
            nc.gpsimd.dma_start(wtile[:], wt[:])
            onert = cpool.tile([1, 128], F16, name="onert")
            nc.gpsimd.dma_start(onert[:], onesrd[:])
            cmapt = cpool.tile([1, 840], F16, name="cmapt")
            nc.gpsimd.dma_start(cmapt[:], cmap16d[:])
            cm3 = cmapt[:].rearrange("p (a c) -> p a c", c=HP)

            xall = xpool.tile([128, NIMG_L * NPIX], F16, name="xall")
            for m in range(NIMG_L):
                nc.sync.dma_start(
                    xall[:, m * NPIX : (m + 1) * NPIX],
                    xh[:, m * NPIX : (m + 1) * NPIX],
                )
            x3 = xall[:].rearrange("p (n a b) -> p n a b", a=HP, b=HP)

            s_sb = spool.tile([128, NIMG_L * NV], F32, name="s_sb")
            sums16 = spool.tile([128, 2 * NIMG_L], F32, name="sums16")
            sumsq = spool.tile([128, NIMG_L], F32, name="sumsq")

            # ---- PE warmup: flip HAM to 8/8 during the input-DMA window ----
            warm = psr.tile([128, 1024], F32, name="warm", tag="r4")
            for i in range(N_WARM):
                nc.tensor.matmul(
                    warm[:, 0:128], idt[:], idt[:], start=True, stop=True,
                    skip_group_check=True,
                )

            # ---- ACT spline-table preload (first activation pays ~1.3us);
            # memset feeds it so it has no input-DMA dependency ----
            tscr = spool.tile([128, 8], F32, name="tscr")
            nc.vector.memset(tscr[:, 0:4], 1.0)
            nc.scalar.activation(
                tscr[:, 4:8], tscr[:, 0:4], mybir.ActivationFunctionType.Square
            )

            # ---- x^2: first two images on DVE (fast startup), rest on ACT;
            # emitted up-front so the ACT FIFO serves them before the drains ----
            uts = []
            for m in range(NIMG_L):
                ut = upool.tile([128, NPIX], F16, name=f"u{m}", tag="u")
                xs = xall[:, m * NPIX : (m + 1) * NPIX]
                if m < 2:
                    nc.vector.tensor_mul(ut[:], xs, xs)
                else:
                    nc.scalar.activation(
                        ut[:], xs, mybir.ActivationFunctionType.Square
                    )
                uts.append(ut)

            tfs = [None] * NIMG_L

            def box_chain(m):
                """r4 matmul + centered cast + separable 3x3 box filter for
                image m; leaves tf (t1 - 128*count on the 30-grid) in tfs[m]."""
                r4 = psr.tile([128, 1024], F32, name=f"r4_{m}", tag="r4")
                for lo, hi in ((0, 512), (512, NPIX)):
                    nc.tensor.matmul(
                        r4[:, lo:hi],
                        ones128[:],
                        uts[m][:, lo:hi],
                        start=True,
                        stop=True,
                        skip_group_check=True,
                    )
                rc = boxp.tile([128, NPIX], F16, name=f"rc{m}", tag="rc")
                nc.vector.tensor_sub(rc[:], r4[:, 0:NPIX], compt[:])
                vv = boxp.tile([128, 840], F16, name=f"vv{m}", tag="vv")
                nc.vector.tensor_add(vv[:], rc[:, 0:840], rc[:, 30:870])
                nc.vector.tensor_add(vv[:], vv[:], rc[:, 60:900])
                te = boxp.tile([128, 840], F16, name=f"te{m}", tag="te")
                nc.vector.tensor_add(te[:, 0:838], vv[:, 0:838], vv[:, 2:840])
                tf = tfp.tile([128, 840], F16, name=f"tf{m}", tag="tf")
                nc.gpsimd.tensor_add(tf[:, 0:420], te[:, 0:420], vv[:, 1:421])
                nc.vector.tensor_add(tf[:, 420:838], te[:, 420:838], vv[:, 421:839])
                tfs[m] = tf

            def conv_chunk(b):
                """Conv accumulation groups + drains for images 2b, 2b+1.
                One psum BANK per (img, yt-half) group: finest-grained drain
                so the next chunk's injects never wait long."""
                ms = (2 * b, 2 * b + 1)
                pss = {}
                for m in ms:
                    for yt in range(2):
                        pss[(m, yt)] = psc.tile(
                            [128, 512], F32, name=f"ps{m}_{yt}", tag="ps"
                        )
                # t1 injection (shared idt weights)
                for m in ms:
                    t13 = tfs[m][:].rearrange("p (a c) -> p a c", c=HP)
                    for yt in range(2):
                        nc.tensor.matmul(
                            pss[(m, yt)][:, 0:392],
                            idt[:],
                            t13[:, 14 * yt : 14 * yt + 14, 0:28],
                            start=True,
                            stop=False,
                            skip_group_check=True,
                        )
                # conv: k-major so each weight load serves 4 matmuls
                for k in range(9):
                    dy, dx = divmod(k, 3)
                    for m in ms:
                        for yt in range(2):
                            y0 = yt * 14
                            nc.tensor.matmul(
                                pss[(m, yt)][:, 0:392],
                                wtile[:, k * 128 : (k + 1) * 128],
                                x3[:, m, y0 + dy : y0 + dy + 14, dx : dx + 28],
                                start=False,
                                stop=False,
                                skip_group_check=True,
                            )
                # countmap (uncenter) + close the groups
                for m in ms:
                    for yt in range(2):
                        nc.tensor.matmul(
                            pss[(m, yt)][:, 0:392],
                            onert[:],
                            cm3[:, 14 * yt : 14 * yt + 14, 0:28],
                            start=False,
                            stop=True,
                            skip_group_check=True,
                        )
                # drains: psum -> s_sb with accum S (per half), then squares
                # with accum Q (per image)
                for m in ms:
                    for yt in range(2):
                        blk = m * NV + yt * 392
                        nc.scalar.activation(
                            s_sb[:, blk : blk + 392],
                            pss[(m, yt)][:, 0:392],
                            mybir.ActivationFunctionType.Copy,
                            accum_out=sums16[:, 2 * m + yt : 2 * m + yt + 1],
                        )
                for m in ms:
                    blk = m * NV
                    sq_scr = opool.tile([128, NV], F32, name=f"sq{m}", tag="sq")
                    nc.scalar.activation(
                        sq_scr[:],
                        s_sb[:, blk : blk + NV],
                        mybir.ActivationFunctionType.Square,
                        accum_out=sumsq[:, m : m + 1],
                    )

            for m in (0, 1, 2, 3):
                box_chain(m)
            conv_chunk(0)
            for m in (4, 5):
                box_chain(m)
            conv_chunk(1)
            for m in (6, 7):
                box_chain(m)
            conv_chunk(2)
            conv_chunk(3)

            # ---- stats: local fold -> 4-rank AllGather -> global fold ----
            st2 = spool.tile([128, 2], F32, name="st2")
            nc.vector.tensor_reduce(
                out=st2[:, 0:1], in_=sums16[:], op=mybir.AluOpType.add,
                axis=mybir.AxisListType.X,
            )
            nc.vector.tensor_reduce(
                out=st2[:, 1:2], in_=sumsq[:], op=mybir.AluOpType.add,
                axis=mybir.AxisListType.X,
            )
            cin = dram.tile([128, 2], F32, name="cin")
            cout = dram.tile([128 * 4, 2], F32, name="cout")
            nc.sync.dma_start(cin[:], st2[:])
            nc.gpsimd.collective_compute(
                "AllGather",
                mybir.AluOpType.bypass,
                replica_groups=CC_GROUPS,
                ins=[cin[:].opt()],
                outs=[cout[:].opt()],
            )
            g = spool.tile([128, 8], F32, name="g")
            nc.sync.dma_start(
                g[:], AP(cout.tensor, cout.offset, [[2, 128], [256, 4], [1, 2]])
            )
            gs = spool.tile([128, 2], F32, name="gs")
            nc.vector.tensor_add(gs[:], g[:, 0:2], g[:, 2:4])
            nc.vector.tensor_add(gs[:], gs[:], g[:, 4:6])
            nc.vector.tensor_add(gs[:], gs[:], g[:, 6:8])

            ab = spool.tile([128, 8], F32, name="ab")
            mean = ab[:, 0:1]
            qn = ab[:, 1:2]
            nc.vector.tensor_scalar_mul(mean, gs[:, 0:1], 1.0 / NHW)
            nc.vector.tensor_scalar_mul(qn, gs[:, 1:2], 1.0 / NHW)
            var = ab[:, 2:3]
            nc.vector.scalar_tensor_tensor(
                var, mean, 1.0, mean, op0=mybir.AluOpType.mult,
                op1=mybir.AluOpType.mult,
            )
            nc.vector.tensor_sub(var, qn, var)
            sd = ab[:, 3:4]
            nc.scalar.activation(
                sd, var, mybir.ActivationFunctionType.Sqrt, bias=c32[:, 2:3]
            )
            abv = spool.tile([128, 2], F32, name="abv")
            A = abv[:, 0:1]
            B = abv[:, 1:2]
            nc.vector.reciprocal(A, sd)
            nc.vector.tensor_mul(A, A, c32[:, 0:1])
            nc.vector.scalar_tensor_tensor(
                B, mean, 1.0, A, op0=mybir.AluOpType.mult, op1=mybir.AluOpType.mult
            )
            nc.vector.tensor_sub(B, c32[:, 1:2], B)

            # ---- normalize + store (engine rotation) ----
            for m in range(NIMG_L):
                blk = m * NV
                o = opool.tile([128, NV], F16, name=f"o{m}", tag="o")
                if m % 2 == 0:
                    nc.vector.tensor_scalar(
                        o[:],
                        s_sb[:, blk : blk + NV],
                        A,
                        B,
                        op0=mybir.AluOpType.mult,
                        op1=mybir.AluOpType.add,
                    )
                else:
                    nc.scalar.activation(
                        o[:],
                        s_sb[:, blk : blk + NV],
                        mybir.ActivationFunctionType.Identity,
                        bias=B,
                        scale=A,
                    )
                dst = AP(y.ap().tensor, m * 128 * NV, [[NV, 128], [1, NV]])
                eng = nc.sync if m % 2 == 0 else nc.scalar
                eng.dma_start(dst, o[:])
    return nc


def _prep_inputs(x, w, gamma, beta):
    x = np.asarray(x, np.float32)
    w = np.asarray(w, np.float32)
    gamma = np.asarray(gamma, np.float32)
    beta = np.asarray(beta, np.float32)

    xp = np.zeros((32, 128, HP, HP), np.float32)
    xp[:, :, 1:29, 1:29] = x

    ones128 = np.ones((128, 128), np.float16)
    id128 = np.eye(128, dtype=np.float16)
    onesr = np.ones((1, 128), np.float16)

    pidx = np.arange(NPIX)
    py, px = pidx // HP, pidx % HP
    valid = (py >= 1) & (py <= 28) & (px >= 1) & (px <= 28)
    comp16 = np.broadcast_to((128.0 * valid).astype(np.float16), (128, NPIX)).copy()

    jj = np.arange(840)
    jy, jx = jj // HP, jj % HP
    cy = np.where((jy == 0) | (jy == 27), 2, 3)
    cx = np.where((jx == 0) | (jx == 27), 2, 3)
    used = (jy < 28) & (jx < 28)
    cmap16 = np.where(used, 128.0 * cy * cx, 0.0).astype(np.float16)[None, :]

    maps = []
    for core in range(N_CORES):
        cg, bg = core // 4, core % 4
        xs = xp[bg * NIMG_L : (bg + 1) * NIMG_L]
        xhc = np.ascontiguousarray(xs.transpose(1, 0, 2, 3)).reshape(
            128, NIMG_L * NPIX
        )
        wc = (2.0 * w[cg * 128 : (cg + 1) * 128]).reshape(128, 128, 9)
        wtc = np.ascontiguousarray(wc.transpose(1, 2, 0)).reshape(128, 9 * 128)
        cst32 = np.zeros((128, 3), np.float32)
        cst32[:, 0] = gamma[cg * 128 : (cg + 1) * 128]
        cst32[:, 1] = beta[cg * 128 : (cg + 1) * 128]
        cst32[:, 2] = EPS
        maps.append(
            {
                "xh": xhc.astype(np.float16),
                "wt": wtc.astype(np.float16),
                "ones128": ones128,
                "id128": id128,
                "onesr": onesr,
                "comp16": comp16,
                "cmap16": cmap16,
                "cst32": cst32,
            }
        )
    return maps


_NC_CACHE = []


def _assemble(results):
    out = np.empty((32, 256, 28, 28), np.float32)
    for core in range(N_CORES):
        cg, bg = core // 4, core % 4
        out[bg * NIMG_L : (bg + 1) * NIMG_L, cg * 128 : (cg + 1) * 128] = (
            results[core]["y"].astype(np.float32)
        )
    return out


def kernel(x, w, gamma, beta):
    if not _NC_CACHE:
        _NC_CACHE.append(_build_nc())
    nc = _NC_CACHE[0]
    maps = _prep_inputs(x, w, gamma, beta)
    res = run_bass_kernel_spmd(nc, maps, core_ids=list(range(N_CORES)))
    return _assemble(res.results)


# revision 10
# speedup vs baseline: 1.1375x; 1.0242x over previous
"""EuclidConv + training-mode BatchNorm on 8 Trainium2 NeuronCores.

Math (reference): out = BN(2*conv(x,w) + conv(x^2, ones3x3) + ||w_f||^2),
BN over global-batch stats. The per-filter ||w||^2 term is channel-constant,
so BN's mean subtraction cancels it exactly -> never computed.

Sharding: HYBRID. core c -> (chgrp = c//4, bgrp = c%4): 128 of 256 output
channels x 8 of 32 images. This gives full-width M=128 matmuls (the pure
channel-sharded layout only fills 32 of 128 PE output columns), 4x less PE
streaming. The price: BN statistics must be reduced across the 4 bgrps that
share a channel group -> one tiny 4-rank AllGather of [128,2] partial
(sum, sumsq) + local fold.

Per image m (padded 30x30 grid, fp16):
  u_m = x_m^2                                      (ACT Square)
  r4 psum = ones128.T @ u_m    (channel sums of x^2, replicated over all
                                128 partitions; 2 MMs)
  rc = r4 - 128*validmap       (DVE, fp16, centered for precision)
  box filter: vv = 3-tap vertical (DVE, stride-30 = pair-aligned 2x mode),
  te = vv[0]+vv[+2] (2x), tf = te + vv[+1] (GpSimd - odd offset would be
  1x-mode on DVE anyway, and DVE is the busier engine)
Conv accumulation group per (img, yt-half) [128,392] psum:
  identity.T @ tf_view         (start=True: seeds psum with t1 - 128*count)
  sum_k (2w)_k.T @ x_view      (9 offsets, full M=128)
  ones1.T @ cmap_view          (stop=True: re-adds 128*count)
Drain: ACT copy psum->s_sb with accum S; ACT square with accum Q.
Stats: fold S,Q over 8 local images -> [128,2]; 4-rank AllGather via HBM
bounce; fold 4 ranks; A = gamma*rsqrt(var+eps), B = beta - mean*A;
normalize out = s*A+B (DVE/ACT/GpSimd rotation, fp16) -> DMA out.

Host-side prep is layout/sharding only: pad+transpose+cast of x, weight
transpose/scale, constant masks.
"""
import json

import numpy as np

import subprocess as _subprocess

import concourse.bass as bass
import concourse.bass_utils as _bass_utils
import concourse.mybir as mybir
import concourse.tile as tile
from concourse.ap import AP
from concourse.bass_utils import run_bass_kernel_spmd
from concourse.vector_clock import ScopedClock, VectorClock

F16 = mybir.dt.float16
F32 = mybir.dt.float32


class _WalrusLdwOpt:
    """Enable walrus's LDWEIGHTS dedup pass for this kernel's compiles.
    bass_utils hardcodes --enable-ldw-opt=false; this kernel issues runs of
    4+ matmuls sharing one stationary operand, where redundant per-matmul
    weight reloads serialize ~110ns each on the PE."""

    def __getattr__(self, name):
        return getattr(_subprocess, name)

    def check_call(self, argv, **kw):
        if (
            isinstance(argv, list)
            and argv
            and "walrus_driver" in str(argv[0])
        ):
            argv = [
                "--enable-ldw-opt=true" if a == "--enable-ldw-opt=false" else a
                for a in argv
            ]
        return _subprocess.check_call(argv, **kw)


_bass_utils.subprocess = _WalrusLdwOpt()

N_CORES = 8
NIMG_L = 8  # images per core
HP = 30
NPIX = HP * HP
NV = 28 * 28
NHW = 32 * NV  # global batch pixels per channel
EPS = 1e-5
CC_GROUPS = [[0, 1, 2, 3], [4, 5, 6, 7]]
N_WARM = 30

_split_ctr = [0]


def _split_waits_json(bir: bytes, max_waits: int = 1) -> bytes:
    """This container's walrus rejects instructions with >1 sync wait.
    Hoist excess waits onto EventSemaphore instructions inserted before the
    offender on the same engine stream."""
    m = json.loads(bir)
    for f in m["functions"]:
        for bb in f["blocks"]:
            newinsts = []
            for ins in bb["instructions"]:
                si = ins.get("sync_info")
                if si:
                    waits = si.get("on_wait") or []
                    if len(waits) > max_waits:
                        extra, keep = waits[:-max_waits], waits[-max_waits:]
                        for w_ in extra:
                            _split_ctr[0] += 1
                            newinsts.append(
                                {
                                    "debug": ins.get("debug", 0),
                                    "engine": ins["engine"],
                                    "ins": [],
                                    "outs": [],
                                    "name": f"antsplitw-{_split_ctr[0]}",
                                    "opcode": "EventSemaphore",
                                    "sync_info": {"on_update": [], "on_wait": [w_]},
                                }
                            )
                        si["on_wait"] = keep
                newinsts.append(ins)
            bb["instructions"] = newinsts
    return json.dumps(m).encode()


class _PatchedBass(bass.Bass):
    def to_json_bytes(self):
        return _split_waits_json(super().to_json_bytes())


class _SplitDrainTileContext(tile.TileContext):
    """Split the tile-exit drain's waits into single-wait drains (same
    walrus limitation as above)."""

    def _drain_and_barrier(self, tick_clock, wait_clock):
        g = tick_clock.global_clock
        n = len(g)
        for i in range(n):
            if g[i] > 0:
                vec = [0] * n
                vec[i] = g[i]
                d = self.nc.sync.drain()
                wait_clock.add_sem_waits(d.ins, ScopedClock({None: VectorClock(vec)}))
        self.nc.sync.drain()
        self.nc.all_engine_barrier()
        assert self.sems is not None
        popped = self.nc._tile_sem_poison_stack.pop()
        assert popped is self._sem_poison
        self.nc.clear_and_free_semaphores(list(self.sems.allocated().values()))
        self.nc.all_engine_barrier()


def _build_nc():
    nc = _PatchedBass(num_devices=N_CORES)
    xh = nc.dram_tensor("xh", [128, NIMG_L * NPIX], F16, kind="ExternalInput")
    wt = nc.dram_tensor("wt", [128, 9 * 128], F16, kind="ExternalInput")
    ones128d = nc.dram_tensor("ones128", [128, 128], F16, kind="ExternalInput")
    id128d = nc.dram_tensor("id128", [128, 128], F16, kind="ExternalInput")
    onesrd = nc.dram_tensor("onesr", [1, 128], F16, kind="ExternalInput")
    comp16d = nc.dram_tensor("comp16", [128, NPIX], F16, kind="ExternalInput")
    cmap16d = nc.dram_tensor("cmap16", [1, 840], F16, kind="ExternalInput")
    cst32d = nc.dram_tensor("cst32", [128, 3], F32, kind="ExternalInput")
    y = nc.dram_tensor("y", [NIMG_L, 128, 28, 28], F16, kind="ExternalOutput")

    with _SplitDrainTileContext(nc) as tc:
        with (
            tc.tile_pool(name="const", bufs=1) as cpool,
            tc.tile_pool(name="xpool", bufs=1) as xpool,
            tc.tile_pool(name="upool", bufs=3) as upool,
            tc.tile_pool(name="boxp", bufs=3) as boxp,
            tc.tile_pool(name="tfp", bufs=4) as tfp,
            tc.tile_pool(name="spool", bufs=1) as spool,
            tc.tile_pool(name="opool", bufs=8) as opool,
            tc.tile_pool(name="psr", bufs=2, space="PSUM") as psr,
            tc.tile_pool(name="psc", bufs=4, space="PSUM") as psc,
            tc.tile_pool(name="dram", bufs=1, space="DRAM") as dram,
        ):
            # ---- dummy collective, triggered first: absorbs the NRT entry
            # barrier + first-collective ncfw setup (~25-50us) under compute,
            # so the real stats AllGather later starts in ~1us ----
            dcin = dram.tile([128, 2], F32, name="dcin")
            dcout = dram.tile([128 * 4, 2], F32, name="dcout")
            nc.gpsimd.collective_compute(
                "AllGather",
                mybir.AluOpType.bypass,
                replica_groups=CC_GROUPS,
                ins=[dcin[:].opt()],
                outs=[dcout[:].opt()],
            )

            # ---- constants: criticality-ordered. Early consumers (warmup,
            # r4, rc) load on the sync queue ahead of the images; bulky /
            # late-consumed ones go on the gpsimd queue ----
            idt = cpool.tile([128, 128], F16, name="idt")
            nc.sync.dma_start(idt[:], id128d[:])
            ones128 = cpool.tile([128, 128], F16, name="ones128")
            nc.sync.dma_start(ones128[:], ones128d[:])
            compt = cpool.tile([128, NPIX], F16, name="compt")
            nc.sync.dma_start(compt[:], comp16d[:])
            c32 = cpool.tile([128, 3], F32, name="c32")
            nc.sync.dma_start(c32[:], cst32d[:])
            wtile = cpool.tile([128, 9 * 128], F16, name="wtile")
            nc.gpsimd.dma_start(wtile[:], wt[:])
            onert = cpool.tile([1, 128], F16, name="onert")
            nc.gpsimd.dma_start(onert[:], onesrd[:])
            cmapt = cpool.tile([1, 840], F16, name="cmapt")
            nc.gpsimd.dma_start(cmapt[:], cmap16d[:])
            cm3 = cmapt[:].rearrange("p (a c) -> p a c", c=HP)

            xall = xpool.tile([128, NIMG_L * NPIX], F16, name="xall")
            for m in range(NIMG_L):
                nc.sync.dma_start(
                    xall[:, m * NPIX : (m + 1) * NPIX],
                    xh[:, m * NPIX : (m + 1) * NPIX],
                )
            x3 = xall[:].rearrange("p (n a b) -> p n a b", a=HP, b=HP)

            s_sb = spool.tile([128, NIMG_L * NV], F32, name="s_sb")
            sums16 = spool.tile([128, 2 * NIMG_L], F32, name="sums16")
            sumsq = spool.tile([128, NIMG_L], F32, name="sumsq")

            # ---- PE warmup: flip HAM to 8/8 during the input-DMA window ----
            warm = psr.tile([128, 1024], F32, name="warm", tag="r4")
            for i in range(N_WARM):
                nc.tensor.matmul(
                    warm[:, 0:128], idt[:], idt[:], start=True, stop=True,
                    skip_group_check=True,
                )

            # ---- ACT spline-table preload (first activation pays ~1.3us);
            # memset feeds it so it has no input-DMA dependency ----
            tscr = spool.tile([128, 8], F32, name="tscr")
            nc.vector.memset(tscr[:, 0:4], 1.0)
            nc.scalar.activation(
                tscr[:, 4:8], tscr[:, 0:4], mybir.ActivationFunctionType.Square
            )

            # ---- x^2: first two images on DVE (fast startup), rest on ACT;
            # emitted up-front so the ACT FIFO serves them before the drains ----
            uts = []
            for m in range(NIMG_L):
                ut = upool.tile([128, NPIX], F16, name=f"u{m}", tag="u")
                xs = xall[:, m * NPIX : (m + 1) * NPIX]
                if m < 2:
                    nc.vector.tensor_mul(ut[:], xs, xs)
                else:
                    nc.scalar.activation(
                        ut[:], xs, mybir.ActivationFunctionType.Square
                    )
                uts.append(ut)

            tfs = [None] * NIMG_L

            def box_chain(m):
                """r4 matmul + centered cast + separable 3x3 box filter for
                image m; leaves tf (t1 - 128*count on the 30-grid) in tfs[m]."""
                r4 = psr.tile([128, 1024], F32, name=f"r4_{m}", tag="r4")
                for lo, hi in ((0, 512), (512, NPIX)):
                    nc.tensor.matmul(
                        r4[:, lo:hi],
                        ones128[:],
                        uts[m][:, lo:hi],
                        start=True,
                        stop=True,
                        skip_group_check=True,
                    )
                rc = boxp.tile([128, NPIX], F16, name=f"rc{m}", tag="rc")
                nc.vector.tensor_sub(rc[:], r4[:, 0:NPIX], compt[:])
                vv = boxp.tile([128, 840], F16, name=f"vv{m}", tag="vv")
                nc.vector.tensor_add(vv[:], rc[:, 0:840], rc[:, 30:870])
                nc.vector.tensor_add(vv[:], vv[:], rc[:, 60:900])
                te = boxp.tile([128, 840], F16, name=f"te{m}", tag="te")
                nc.vector.tensor_add(te[:, 0:838], vv[:, 0:838], vv[:, 2:840])
                tf = tfp.tile([128, 840], F16, name=f"tf{m}", tag="tf")
                nc.gpsimd.tensor_add(tf[:, 0:420], te[:, 0:420], vv[:, 1:421])
                nc.vector.tensor_add(tf[:, 420:838], te[:, 420:838], vv[:, 421:839])
                tfs[m] = tf

            def conv_chunk(b):
                """Conv accumulation groups + drains for images 2b, 2b+1.
                One psum BANK per (img, yt-half) group: finest-grained drain
                so the next chunk's injects never wait long."""
                ms = (2 * b, 2 * b + 1)
                pss = {}
                for m in ms:
                    for yt in range(2):
                        pss[(m, yt)] = psc.tile(
                            [128, 512], F32, name=f"ps{m}_{yt}", tag="ps"
                        )
                # t1 injection (shared idt weights)
                for m in ms:
                    t13 = tfs[m][:].rearrange("p (a c) -> p a c", c=HP)
                    for yt in range(2):
                        nc.tensor.matmul(
                            pss[(m, yt)][:, 0:392],
                            idt[:],
                            t13[:, 14 * yt : 14 * yt + 14, 0:28],
                            start=True,
                            stop=False,
                            skip_group_check=True,
                        )
                # conv: k-major so each weight load serves 4 matmuls
                for k in range(9):
                    dy, dx = divmod(k, 3)
                    for m in ms:
                        for yt in range(2):
                            y0 = yt * 14
                            nc.tensor.matmul(
                                pss[(m, yt)][:, 0:392],
                                wtile[:, k * 128 : (k + 1) * 128],
                                x3[:, m, y0 + dy : y0 + dy + 14, dx : dx + 28],
                                start=False,
                                stop=False,
                                skip_group_check=True,
                            )
                # countmap (uncenter) + close the groups
                for m in ms:
                    for yt in range(2):
                        nc.tensor.matmul(
                            pss[(m, yt)][:, 0:392],
                            onert[:],
                            cm3[:, 14 * yt : 14 * yt + 14, 0:28],
                            start=False,
                            stop=True,
                            skip_group_check=True,
                        )
                # drains: psum -> s_sb with accum S (per half), then squares
                # with accum Q (per image)
                for m in ms:
                    for yt in range(2):
                        blk = m * NV + yt * 392
                        nc.scalar.activation(
                            s_sb[:, blk : blk + 392],
                            pss[(m, yt)][:, 0:392],
                            mybir.ActivationFunctionType.Copy,
                            accum_out=sums16[:, 2 * m + yt : 2 * m + yt + 1],
                        )
                for m in ms:
                    blk = m * NV
                    sq_scr = opool.tile([128, NV], F32, name=f"sq{m}", tag="sq")
                    nc.scalar.activation(
                        sq_scr[:],
                        s_sb[:, blk : blk + NV],
                        mybir.ActivationFunctionType.Square,
                        accum_out=sumsq[:, m : m + 1],
                    )

            for m in (0, 1, 2, 3):
                box_chain(m)
            conv_chunk(0)
            for m in (4, 5):
                box_chain(m)
            conv_chunk(1)
            for m in (6, 7):
                box_chain(m)
            conv_chunk(2)
            conv_chunk(3)

            # ---- stats: local fold -> 4-rank AllGather -> global fold ----
            st2 = spool.tile([128, 2], F32, name="st2")
            nc.vector.tensor_reduce(
                out=st2[:, 0:1], in_=sums16[:], op=mybir.AluOpType.add,
                axis=mybir.AxisListType.X,
            )
            nc.vector.tensor_reduce(
                out=st2[:, 1:2], in_=sumsq[:], op=mybir.AluOpType.add,
                axis=mybir.AxisListType.X,
            )
            cin = dram.tile([128, 2], F32, name="cin")
            cout = dram.tile([128 * 4, 2], F32, name="cout")
            nc.sync.dma_start(cin[:], st2[:])
            nc.gpsimd.collective_compute(
                "AllGather",
                mybir.AluOpType.bypass,
                replica_groups=CC_GROUPS,
                ins=[cin[:].opt()],
                outs=[cout[:].opt()],
            )
            g = spool.tile([128, 8], F32, name="g")
            nc.sync.dma_start(
                g[:], AP(cout.tensor, cout.offset, [[2, 128], [256, 4], [1, 2]])
            )
            gs = spool.tile([128, 2], F32, name="gs")
            nc.vector.tensor_add(gs[:], g[:, 0:2], g[:, 2:4])
            nc.vector.tensor_add(gs[:], gs[:], g[:, 4:6])
            nc.vector.tensor_add(gs[:], gs[:], g[:, 6:8])

            ab = spool.tile([128, 8], F32, name="ab")
            mean = ab[:, 0:1]
            qn = ab[:, 1:2]
            nc.vector.tensor_scalar_mul(mean, gs[:, 0:1], 1.0 / NHW)
            nc.vector.tensor_scalar_mul(qn, gs[:, 1:2], 1.0 / NHW)
            var = ab[:, 2:3]
            nc.vector.scalar_tensor_tensor(
                var, mean, 1.0, mean, op0=mybir.AluOpType.mult,
                op1=mybir.AluOpType.mult,
            )
            nc.vector.tensor_sub(var, qn, var)
            sd = ab[:, 3:4]
            nc.scalar.activation(
                sd, var, mybir.ActivationFunctionType.Sqrt, bias=c32[:, 2:3]
            )
            abv = spool.tile([128, 2], F32, name="abv")
            A = abv[:, 0:1]
            B = abv[:, 1:2]
            nc.vector.reciprocal(A, sd)
            nc.vector.tensor_mul(A, A, c32[:, 0:1])
            nc.vector.scalar_tensor_tensor(
                B, mean, 1.0, A, op0=mybir.AluOpType.mult, op1=mybir.AluOpType.mult
            )
            nc.vector.tensor_sub(B, c32[:, 1:2], B)

            # ---- normalize + store (engine rotation) ----
            for m in range(NIMG_L):
                blk = m * NV
                o = opool.tile([128, NV], F16, name=f"o{m}", tag="o")
                if m % 2 == 0:
                    nc.vector.tensor_scalar(
                        o[:],
                        s_sb[:, blk : blk + NV],
                        A,
                        B,
                        op0=mybir.AluOpType.mult,
                        op1=mybir.AluOpType.add,
                    )
                else:
                    nc.scalar.activation(
                        o[:],
                        s_sb[:, blk : blk + NV],
                        mybir.ActivationFunctionType.Identity,
                        bias=B,
                        scale=A,
                    )
                dst = AP(y.ap().tensor, m * 128 * NV, [[NV, 128], [1, NV]])
                eng = nc.sync if m % 2 == 0 else nc.scalar
                eng.dma_start(dst, o[:])
    return nc


def _prep_inputs(x, w, gamma, beta):
    x = np.asarray(x, np.float32)
    w = np.asarray(w, np.float32)
    gamma = np.asarray(gamma, np.float32)
    beta = np.asarray(beta, np.float32)

    xp = np.zeros((32, 128, HP, HP), np.float32)
    xp[:, :, 1:29, 1:29] = x

    ones128 = np.ones((128, 128), np.float16)
    id128 = np.eye(128, dtype=np.float16)
    onesr = np.ones((1, 128), np.float16)

    pidx = np.arange(NPIX)
    py, px = pidx // HP, pidx % HP
    valid = (py >= 1) & (py <= 28) & (px >= 1) & (px <= 28)
    comp16 = np.broadcast_to((128.0 * valid).astype(np.float16), (128, NPIX)).copy()

    jj = np.arange(840)
    jy, jx = jj // HP, jj % HP
    cy = np.where((jy == 0) | (jy == 27), 2, 3)
    cx = np.where((jx == 0) | (jx == 27), 2, 3)
    used = (jy < 28) & (jx < 28)
    cmap16 = np.where(used, 128.0 * cy * cx, 0.0).astype(np.float16)[None, :]

    maps = []
    for core in range(N_CORES):
        cg, bg = core // 4, core % 4
        xs = xp[bg * NIMG_L : (bg + 1) * NIMG_L]
        xhc = np.ascontiguousarray(xs.transpose(1, 0, 2, 3)).reshape(
            128, NIMG_L * NPIX
        )
        wc = (2.0 * w[cg * 128 : (cg + 1) * 128]).reshape(128, 128, 9)
        wtc = np.ascontiguousarray(wc.transpose(1, 2, 0)).reshape(128, 9 * 128)
        cst32 = np.zeros((128, 3), np.float32)
        cst32[:, 0] = gamma[cg * 128 : (cg + 1) * 128]
        cst32[:, 1] = beta[cg * 128 : (cg + 1) * 128]
        cst32[:, 2] = EPS
        maps.append(
            {
                "xh": xhc.astype(np.float16),
                "wt": wtc.astype(np.float16),
                "ones128": ones128,
                "id128": id128,
                "onesr": onesr,
                "comp16": comp16,
                "cmap16": cmap16,
                "cst32": cst32,
            }
        )
    return maps


_NC_CACHE = []


def _assemble(results):
    out = np.empty((32, 256, 28, 28), np.float32)
    for core in range(N_CORES):
        cg, bg = core // 4, core % 4
        out[bg * NIMG_L : (bg + 1) * NIMG_L, cg * 128 : (cg + 1) * 128] = (
            results[core]["y"].astype(np.float32)
        )
    return out


def kernel(x, w, gamma, beta):
    if not _NC_CACHE:
        _NC_CACHE.append(_build_nc())
    nc = _NC_CACHE[0]
    maps = _prep_inputs(x, w, gamma, beta)
    res = run_bass_kernel_spmd(nc, maps, core_ids=list(range(N_CORES)))
    return _assemble(res.results)
